# revision 38
# baseline (speedup 1.0000x reference)
"""Capsule-routing kernel for Trainium2, 8-core batch-parallel.

Reference computation (per example, In=4096, D=256, N=16, K=16, routings=3):
    u_hat = (x @ W).reshape(In, N, K)           # [In, 256] with m = n*16+k
    b = 0
    for j in range(3):
        c = softmax(b, axis=n)                   # [In, N]
        outputs = squash(sum_i c[i,n] u_hat[i,n,:])   # [N, K]
        if j < 2: b[i,n] = sum_k outputs[n,k] u_hat[i,n,k]

Key algebraic restructuring: u_hat is NEVER materialized.
  - outputs accumulation:  acc = (c^T x) W = yT^T @ W  with
    yT[d,n] = sum_i x[i,d] c[i,n]  (64 matmuls of 16-free per round)
  - b update:  b = (x W) S = x @ (W S)  with WS [256,16] built by 4 tiny
    matmuls from S = (masked outputs)^T scaled by rinv.
  - round 0 (c uniform 1/16): yT0 = colsum(x)/16 broadcast, where
    colsum accumulates via 1-col matmuls folded into the transpose phase.
  - squash: om = acc*mask is transposed on PE; Square runs on the
    128-partition form (32-free); nrm2 via PE matmul with ones;
    rinv = exp(-0.5*ln(nrm2+eps)) so ScalarE stays on ONE ACT table
    (natural_log_exp_and_others: Copy/Square/Exp/Ln).
x is shipped bf16 i-major; xT (lhsT for the b update) is built on
device with PE transposes. Everything runs in bf16 (tol 2e-2).
"""

import sys
from contextlib import ExitStack

sys.path.insert(0, "/opt/trn_rl_repo")

import numpy as np
import ml_dtypes

import concourse.bass as bass
import concourse.mybir as mybir
import concourse.tile as tile
from concourse import bacc
from concourse.bass_utils import run_bass_kernel_spmd

# All ScalarE funcs used here (Copy/Square/Exp/Ln) live together in the
# natural_log_exp_and_others ACT table. Put it first so the act-table
# insertion pass resolves every activation to that one table instead of
# thrashing between exp_and_others and natural_log (1283 ns per reload).
from concourse.hw_specs import get_activation_tables as _gat_orig


def _gat_pref(arch):
    t = _gat_orig(arch)
    pref = "natural_log_exp_and_others"
    if pref in t:
        return {pref: t[pref],
                **{k: v for k, v in t.items() if k != pref}}
    return t


bacc.get_activation_tables = _gat_pref

F32 = mybir.dt.float32
BF16 = mybir.dt.bfloat16
AF = mybir.ActivationFunctionType

N_CORES = 8
B = 32
IN = 4096
D = 256
N = 16
K = 16
M = N * K  # 256
EPS = 1e-7
N_EX = 4
N_T = 32  # i tiles of 128


def build_kernel():
    nc = bacc.Bacc("TRN2", target_bir_lowering=False, debug=False,
                   num_devices=N_CORES)

    x_d = nc.dram_tensor("x", [N_EX, 128, N_T, D], BF16, kind="ExternalInput")
    Wt_d = nc.dram_tensor("Wt", [128, 2, M], BF16, kind="ExternalInput")
    WtT_d = nc.dram_tensor("WtT", [128, 2, D], BF16, kind="ExternalInput")
    id128_d = nc.dram_tensor("id128", [128, 128], BF16, kind="ExternalInput")
    ones_d = nc.dram_tensor("ones128", [128, 1], BF16, kind="ExternalInput")
    bmask_d = nc.dram_tensor("bmask", [N, M], BF16, kind="ExternalInput")
    bmaskT_d = nc.dram_tensor("bmaskT", [128, 2, N], BF16,
                              kind="ExternalInput")
    out_d = nc.dram_tensor("out", [N_EX, N, K], F32, kind="ExternalOutput")

    with tile.TileContext(nc) as tc, ExitStack() as ctx:
        const_pool = ctx.enter_context(tc.tile_pool(name="consts", bufs=1))
        x_pool = ctx.enter_context(tc.tile_pool(name="x", bufs=4))
        xT_pool = ctx.enter_context(tc.tile_pool(name="xT", bufs=3))
        c_pool = ctx.enter_context(tc.tile_pool(name="c", bufs=4))
        small_pool = ctx.enter_context(tc.tile_pool(name="small", bufs=4))

        ps_t = ctx.enter_context(tc.tile_pool(name="ps_t", bufs=2, space="PSUM"))
        ps_b = ctx.enter_context(tc.tile_pool(name="ps_b", bufs=2, space="PSUM"))
        ps_m = ctx.enter_context(tc.tile_pool(name="ps_m", bufs=4, space="PSUM"))

        # ---- constants ----
        Wt = const_pool.tile([128, 2, M], BF16, tag="Wt")
        nc.sync.dma_start(Wt[:], Wt_d[:])
        WtT = const_pool.tile([128, 2, D], BF16, tag="WtT")
        nc.sync.dma_start(WtT[:], WtT_d[:])
        id128 = const_pool.tile([128, 128], BF16, tag="id128")
        nc.sync.dma_start(id128[:], id128_d[:])
        ones128 = const_pool.tile([128, 1], BF16, tag="ones128")
        nc.sync.dma_start(ones128[:], ones_d[:])
        onesrow = const_pool.tile([1, 128], BF16, tag="onesrow")
        nc.sync.dma_start(onesrow[:], ones_d.ap().rearrange("p o -> o p"))
        bmask = const_pool.tile([N, M], BF16, tag="bmask")
        nc.sync.dma_start(bmask[:], bmask_d[:])
        bmaskT = const_pool.tile([128, 2, N], BF16, tag="bmaskT")
        nc.sync.dma_start(bmaskT[:], bmaskT_d[:])
        out_stage = const_pool.tile([N, N_EX, K], F32, tag="out_stage")
        eps_t = const_pool.tile([N, 1], F32, tag="eps")
        nc.vector.memset(eps_t[:], EPS)
        eps_r = const_pool.tile([1, 1], F32, tag="eps_r")
        nc.vector.memset(eps_r[:], EPS)

        # ---- x loads (prefetch, chunked so compute can start early) ----
        x_tiles = []
        for e in range(N_EX):
            xs = x_pool.tile([128, N_T, D], BF16, tag="x")
            for q in range(4):
                nc.sync.dma_start(xs[:, 8 * q:8 * (q + 1), :],
                                  x_d[e, :, 8 * q:8 * (q + 1), :])
            x_tiles.append(xs)

        st = [dict() for _ in range(N_EX)]

        # one shared PSUM bank subdivided for the small per-round tiles.
        # layout (f32 columns): yT @0..32, accT @32..64, rbc @64..96,
        # wsps @96..128, nrow @128..144, acc(j2 only) @224..480 (parts 0..15)
        def misc_views():
            m = ps_m.tile([128, 480], F32, tag="misc")
            yT_ps = m[:, 0:32].rearrange("p (c n) -> p c n", n=N)
            accT_ps = m[:, 32:64].rearrange("p (c n) -> p c n", n=N)
            rbc_ps = m[:, 64:96].rearrange("p (c n) -> p c n", n=N)
            wsps = m[:, 96:128].rearrange("p (c n) -> p c n", n=N)
            nrow_ps = m[0:1, 128:144]
            acc_ps = m[0:N, 224:480]
            return yT_ps, accT_ps, rbc_ps, wsps, nrow_ps, acc_ps

        # big-copy engine rotation: P7 V5 A4 per 16
        cp_engines = [nc.gpsimd, nc.vector, nc.scalar, nc.gpsimd,
                      nc.vector, nc.gpsimd, nc.scalar, nc.vector,
                      nc.gpsimd, nc.gpsimd, nc.vector, nc.scalar,
                      nc.gpsimd, nc.vector, nc.gpsimd, nc.scalar]

        def phase_T(e):
            """Build xT[d, i] via PE transposes; fold in colsum matmuls
            (round-0 shortcut: c uniform -> yT0 = colsum(x)/16)."""
            xs = x_tiles[e]
            xT = xT_pool.tile([128, 2, IN], BF16, tag="xT")
            cs = misc_views()
            cs_ps = cs[0]  # yT slot of this misc buf
            for tp in range(N_T // 2):
                psT = ps_t.tile([128, 2, 2, 128], BF16, tag="psT")
                for ti in range(2):
                    t = 2 * tp + ti
                    for dc in range(2):
                        nc.tensor.transpose(
                            psT[:, dc, ti, :],
                            xs[:, t, 128 * dc:128 * (dc + 1)], id128[:])
                        nc.tensor.matmul(
                            cs_ps[:, dc, 0:1],
                            xs[:, t, 128 * dc:128 * (dc + 1)], ones128[:],
                            start=(t == 0), stop=(t == N_T - 1),
                            skip_group_check=True)
                eng = cp_engines[tp]
                dst = xT[:, :, 256 * tp:256 * (tp + 1)]
                if eng is nc.vector:
                    eng.tensor_copy(dst, psT[:])
                elif eng is nc.scalar:
                    eng.copy(dst.bitcast(F32), psT[:].bitcast(F32))
                else:
                    eng.tensor_copy(dst.bitcast(F32), psT[:].bitcast(F32))
            st[e]["xT"] = xT
            st[e]["cs_ps"] = cs_ps

        def round_stages(e, j):
            """Return the list of stage-closures for routing round (e, j).

            Emitted stage-lockstep across the examples of a diagonal so no
            engine stream has long single-example runs (head-of-line
            blocking across the in-order engine queues)."""
            xT = st[e]["xT"]
            xs = x_tiles[e]
            ctxd = {}

            def s_yT():
                v = misc_views()
                ctxd["v"] = v
                yT_ps = v[0]
                if j == 0:
                    yT_sb = small_pool.tile([128, 2, N], BF16, tag="yTsb")
                    nc.vector.tensor_scalar_mul(
                        yT_sb[:],
                        st[e]["cs_ps"][:, :, 0:1].to_broadcast([128, 2, N]),
                        1.0 / N)
                    ctxd["yT_sb"] = yT_sb
                else:
                    c_all = st[e]["c_all"]
                    for t in range(N_T):
                        for dc in range(2):
                            nc.tensor.matmul(
                                yT_ps[:, dc, :],
                                xs[:, t, 128 * dc:128 * (dc + 1)],
                                c_all[:, t, :],
                                start=(t == 0), stop=(t == N_T - 1),
                                skip_group_check=True)

            def s_yTsb():
                if j != 0:
                    yT_sb = small_pool.tile([128, 2, N], BF16, tag="yTsb")
                    nc.gpsimd.tensor_copy(yT_sb[:], ctxd["v"][0])
                    ctxd["yT_sb"] = yT_sb

            if j == 2:
                def s_acc():
                    acc_ps = ctxd["v"][5]
                    yT_sb = ctxd["yT_sb"]
                    for dc in range(2):
                        nc.tensor.matmul(acc_ps[:], yT_sb[:, dc, :],
                                         Wt[:, dc, :],
                                         start=(dc == 0), stop=(dc == 1),
                                         skip_group_check=True)

                def s_om():
                    om = small_pool.tile([N, M], BF16, tag="om")
                    nc.gpsimd.tensor_mul(om[:], ctxd["v"][5], bmask[:])
                    ctxd["om"] = om

                def s_sq():
                    sq = small_pool.tile([N, M], BF16, tag="sq")
                    nrm2 = small_pool.tile([N, 1], F32, tag="nrm2")
                    nc.scalar.activation(sq[:], ctxd["om"][:], AF.Square,
                                         accum_out=nrm2[:])
                    ctxd["nrm2"] = nrm2

                def s_lnexp():
                    lnv = small_pool.tile([N, 1], F32, tag="lnv")
                    nc.scalar.activation(lnv[:], ctxd["nrm2"][:], AF.Ln,
                                         bias=eps_t[:])
                    rinv = small_pool.tile([N, 1], F32, tag="rinv")
                    nc.scalar.activation(rinv[:], lnv[:], AF.Exp, scale=-0.5)
                    ctxd["rinv"] = rinv

                def s_out():
                    o_c = small_pool.tile([N, K], F32, tag="o_c")
                    nc.vector.tensor_reduce(
                        o_c[:],
                        ctxd["om"][:].rearrange("p (g k) -> p k g", k=K),
                        axis=mybir.AxisListType.X, op=mybir.AluOpType.add)
                    nc.vector.tensor_scalar_mul(out_stage[:, e, :], o_c[:],
                                                ctxd["rinv"][:])

                return [s_yT, s_yTsb, s_acc, s_om, s_sq, s_lnexp, s_out]

            def s_accT():
                accT_ps = ctxd["v"][1]
                yT_sb = ctxd["yT_sb"]
                for mc in range(2):
                    for dc in range(2):
                        nc.tensor.matmul(
                            accT_ps[:, mc, :],
                            Wt[:, dc, 128 * mc:128 * (mc + 1)],
                            yT_sb[:, dc, :],
                            start=(dc == 0), stop=(dc == 1),
                            skip_group_check=True)

            def s_omT():
                omT = small_pool.tile([128, 2, N], BF16, tag="omT")
                nc.gpsimd.tensor_mul(omT[:], ctxd["v"][1], bmaskT[:])
                ctxd["omT"] = omT

            def s_sqT():
                sqT = small_pool.tile([128, 2, N], BF16, tag="sqT")
                nc.vector.tensor_mul(sqT[:], ctxd["omT"][:], ctxd["omT"][:])
                nrow_ps = ctxd["v"][4]
                for mc in range(2):
                    nc.tensor.matmul(nrow_ps[:], ones128[:], sqT[:, mc, :],
                                     start=(mc == 0), stop=(mc == 1),
                                     skip_group_check=True)
                # WS_raw runs in parallel with the rinv chain
                wsps = ctxd["v"][3]
                omT = ctxd["omT"]
                for dc in range(2):
                    for mc in range(2):
                        nc.tensor.matmul(
                            wsps[:, dc, :],
                            WtT[:, mc, 128 * dc:128 * (dc + 1)],
                            omT[:, mc, :],
                            start=(mc == 0), stop=(mc == 1),
                            skip_group_check=True)

            def s_lnexp():
                lnr = small_pool.tile([1, N], F32, tag="lnr")
                nc.scalar.activation(lnr[:], ctxd["v"][4], AF.Ln,
                                     bias=eps_r[:])
                rrow = small_pool.tile([1, N], BF16, tag="rrow")
                nc.scalar.activation(rrow[:], lnr[:], AF.Exp, scale=-0.5)
                rbc_ps = ctxd["v"][2]
                for mc in range(2):
                    nc.tensor.matmul(rbc_ps[:, mc, :], onesrow[:], rrow[:],
                                     start=True, stop=True,
                                     skip_group_check=True)

            def s_ws():
                ws = small_pool.tile([128, 2, N], BF16, tag="ws")
                nc.vector.tensor_mul(ws[:], ctxd["v"][3], ctxd["v"][2])
                ctxd["ws"] = ws

            def s_b():
                b_ps = ps_b.tile([128, N_T, N], F32, tag="b")
                ws = ctxd["ws"]
                for t in range(N_T):
                    for dc in range(2):
                        nc.tensor.matmul(
                            b_ps[:, t, :],
                            xT[:, dc, 128 * t:128 * (t + 1)], ws[:, dc, :],
                            start=(dc == 0), stop=(dc == 1),
                            skip_group_check=True)
                ctxd["b_ps"] = b_ps
                ctxd["e_all"] = c_pool.tile([128, N_T, N], BF16, tag="e_all", name="e_all")
                ctxd["s_sum"] = c_pool.tile([128, N_T], F32, tag="s_sum", name="s_sum")
                ctxd["s_r"] = c_pool.tile([128, N_T], F32, tag="s_r", name="s_r")
                ctxd["c_new"] = c_pool.tile([128, N_T, N], BF16, tag="c_all", name="c_new")
                st[e]["c_all"] = ctxd["c_new"]

            def softmax_half(h):
                H = N_T // 2
                sl = slice(H * h, H * (h + 1))
                nc.scalar.activation(ctxd["e_all"][:, sl, :],
                                     ctxd["b_ps"][:, sl, :], AF.Exp)
                nc.vector.tensor_reduce(ctxd["s_sum"][:, sl],
                                        ctxd["e_all"][:, sl, :],
                                        axis=mybir.AxisListType.X,
                                        op=mybir.AluOpType.add)
                nc.vector.reciprocal(ctxd["s_r"][:, sl], ctxd["s_sum"][:, sl])
                nc.vector.tensor_mul(
                    ctxd["c_new"][:, sl, :], ctxd["e_all"][:, sl, :],
                    ctxd["s_r"][:, sl].to_broadcast([128, H, N]))

            return [s_yT, s_yTsb, s_accT, s_omT, s_sqT, s_lnexp, s_ws, s_b,
                    lambda: softmax_half(0), lambda: softmax_half(1)]

        def emit_diagonal(members):
            """members: list of (e, j), earliest-data-ready first."""
            stage_lists = [round_stages(e, j) for e, j in members]
            nst = max(len(s) for s in stage_lists)
            for si in range(nst):
                for sl in stage_lists:
                    if si < len(sl):
                        sl[si]()

        phase_T(0)
        phase_T(1)
        emit_diagonal([(0, 0)])
        phase_T(2)
        emit_diagonal([(0, 1), (1, 0)])
        phase_T(3)
        emit_diagonal([(0, 2), (1, 1), (2, 0)])
        emit_diagonal([(1, 2), (2, 1), (3, 0)])
        emit_diagonal([(2, 2), (3, 1)])
        emit_diagonal([(3, 2)])

        nc.sync.dma_start(out_d.ap().rearrange("e n k -> n e k"), out_stage[:])

    nc.compile()
    return nc


_NC_CACHE = {}


def _get_nc():
    if "nc" not in _NC_CACHE:
        _NC_CACHE["nc"] = build_kernel()
    return _NC_CACHE["nc"]


def make_const_inputs():
    bf = ml_dtypes.bfloat16
    id128 = np.eye(128, dtype=bf)
    ones128 = np.ones((128, 1), dtype=bf)
    bmask = np.zeros((N, M), dtype=np.float32)
    for n in range(N):
        bmask[n, n * K:(n + 1) * K] = 1.0
    # bmaskT[p, mc, n] = bmask[n, mc*128 + p]
    bmaskT = np.ascontiguousarray(
        bmask.T.reshape(2, 128, N).transpose(1, 0, 2))
    return id128, ones128, bmask.astype(bf), bmaskT.astype(bf)


def kernel(x, W, num_capsule=None, dim_capsule=None, routings=None, **_):
    bf = ml_dtypes.bfloat16
    x = np.asarray(x, dtype=np.float32)
    W = np.asarray(W, dtype=np.float32)
    assert x.shape == (B, IN, D), x.shape

    nc = _get_nc()
    id128, ones128, bmask, bmaskT = make_const_inputs()
    W0 = W[0]
    Wt = np.ascontiguousarray(
        W0.reshape(2, 128, M).transpose(1, 0, 2)).astype(bf)
    WtT = np.ascontiguousarray(
        W0.T.reshape(2, 128, D).transpose(1, 0, 2)).astype(bf)

    # x[b, i, d] -> [core, e, p, t, d] with i = t*128 + p
    xr = np.ascontiguousarray(
        x.reshape(N_CORES, N_EX, N_T, 128, D).transpose(0, 1, 3, 2, 4)
    ).astype(bf)

    in_maps = []
    for c in range(N_CORES):
        in_maps.append({"x": xr[c], "Wt": Wt, "WtT": WtT, "id128": id128,
                        "ones128": ones128, "bmask": bmask,
                        "bmaskT": bmaskT})

    res = run_bass_kernel_spmd(nc, in_maps, core_ids=list(range(N_CORES)))
    out = np.concatenate([r["out"] for r in res.results], axis=0)
    return out.astype(np.float32)


# revision 39
# speedup vs baseline: 1.0573x; 1.0573x over previous
"""Capsule-routing kernel for Trainium2, 8-core batch-parallel.

Reference computation (per example, In=4096, D=256, N=16, K=16, routings=3):
    u_hat = (x @ W).reshape(In, N, K)           # [In, 256] with m = n*16+k
    b = 0
    for j in range(3):
        c = softmax(b, axis=n)                   # [In, N]
        outputs = squash(sum_i c[i,n] u_hat[i,n,:])   # [N, K]
        if j < 2: b[i,n] = sum_k outputs[n,k] u_hat[i,n,k]

Key algebraic restructuring: u_hat is NEVER materialized.
  - outputs accumulation:  acc = (c^T x) W = yT^T @ W  with
    yT[d,n] = sum_i x[i,d] c[i,n]  (64 matmuls of 16-free per round)
  - b update:  b = (x W) S = x @ (W S)  with WS [256,16] built by 4 tiny
    matmuls from S = (masked outputs)^T scaled by rinv.
  - round 0 (c uniform 1/16): yT0 = colsum(x)/16 broadcast, where
    colsum accumulates via 1-col matmuls folded into the transpose phase.
  - squash: om = acc*mask is transposed on PE; Square runs on the
    128-partition form (32-free); nrm2 via PE matmul with ones;
    rinv = exp(-0.5*ln(nrm2+eps)) so ScalarE stays on ONE ACT table
    (natural_log_exp_and_others: Copy/Square/Exp/Ln).
x is shipped bf16 i-major; xT (lhsT for the b update) is built on
device with PE transposes. Everything runs in bf16 (tol 2e-2).
"""

import sys
from contextlib import ExitStack

sys.path.insert(0, "/opt/trn_rl_repo")

import numpy as np
import ml_dtypes

import concourse.bass as bass
import concourse.mybir as mybir
import concourse.tile as tile
from concourse import bacc
from concourse.bass_utils import run_bass_kernel_spmd

# All ScalarE funcs used here (Copy/Square/Exp/Ln) live together in the
# natural_log_exp_and_others ACT table. Put it first so the act-table
# insertion pass resolves every activation to that one table instead of
# thrashing between exp_and_others and natural_log (1283 ns per reload).
from concourse.hw_specs import get_activation_tables as _gat_orig


def _gat_pref(arch):
    t = _gat_orig(arch)
    pref = "natural_log_exp_and_others"
    if pref in t:
        return {pref: t[pref],
                **{k: v for k, v in t.items() if k != pref}}
    return t


bacc.get_activation_tables = _gat_pref

F32 = mybir.dt.float32
BF16 = mybir.dt.bfloat16
AF = mybir.ActivationFunctionType

N_CORES = 8
B = 32
IN = 4096
D = 256
N = 16
K = 16
M = N * K  # 256
EPS = 1e-7
N_EX = 4
N_T = 32  # i tiles of 128


def build_kernel():
    nc = bacc.Bacc("TRN2", target_bir_lowering=False, debug=False,
                   num_devices=N_CORES)

    x_d = nc.dram_tensor("x", [N_EX, 128, N_T, D], BF16, kind="ExternalInput")
    Wt_d = nc.dram_tensor("Wt", [128, 2, M], BF16, kind="ExternalInput")
    WtT_d = nc.dram_tensor("WtT", [128, 2, D], BF16, kind="ExternalInput")
    id128_d = nc.dram_tensor("id128", [128, 128], BF16, kind="ExternalInput")
    ones_d = nc.dram_tensor("ones128", [128, 1], BF16, kind="ExternalInput")
    bmask_d = nc.dram_tensor("bmask", [N, M], BF16, kind="ExternalInput")
    bmaskT_d = nc.dram_tensor("bmaskT", [128, 2, N], BF16,
                              kind="ExternalInput")
    out_d = nc.dram_tensor("out", [N_EX, N, K], F32, kind="ExternalOutput")

    with tile.TileContext(nc) as tc, ExitStack() as ctx:
        const_pool = ctx.enter_context(tc.tile_pool(name="consts", bufs=1))
        x_pool = ctx.enter_context(tc.tile_pool(name="x", bufs=4))
        xT_pool = ctx.enter_context(tc.tile_pool(name="xT", bufs=3))
        c_pool = ctx.enter_context(tc.tile_pool(name="c", bufs=4))
        small_pool = ctx.enter_context(tc.tile_pool(name="small", bufs=4))

        ps_t = ctx.enter_context(tc.tile_pool(name="ps_t", bufs=2, space="PSUM"))
        ps_b = ctx.enter_context(tc.tile_pool(name="ps_b", bufs=2, space="PSUM"))
        ps_m = ctx.enter_context(tc.tile_pool(name="ps_m", bufs=4, space="PSUM"))

        # ---- constants ----
        Wt = const_pool.tile([128, 2, M], BF16, tag="Wt")
        nc.sync.dma_start(Wt[:], Wt_d[:])
        WtT = const_pool.tile([128, 2, D], BF16, tag="WtT")
        nc.sync.dma_start(WtT[:], WtT_d[:])
        id128 = const_pool.tile([128, 128], BF16, tag="id128")
        nc.sync.dma_start(id128[:], id128_d[:])
        ones128 = const_pool.tile([128, 1], BF16, tag="ones128")
        nc.sync.dma_start(ones128[:], ones_d[:])
        onesrow = const_pool.tile([1, 128], BF16, tag="onesrow")
        nc.sync.dma_start(onesrow[:], ones_d.ap().rearrange("p o -> o p"))
        bmask = const_pool.tile([N, M], BF16, tag="bmask")
        nc.sync.dma_start(bmask[:], bmask_d[:])
        bmaskT = const_pool.tile([128, 2, N], BF16, tag="bmaskT")
        nc.sync.dma_start(bmaskT[:], bmaskT_d[:])
        out_stage = const_pool.tile([N, N_EX, K], F32, tag="out_stage")
        eps_t = const_pool.tile([N, 1], F32, tag="eps")
        nc.vector.memset(eps_t[:], EPS)
        eps_r = const_pool.tile([1, 1], F32, tag="eps_r")
        nc.vector.memset(eps_r[:], EPS)

        # ---- x loads (prefetch, chunked so compute can start early) ----
        x_tiles = []
        for e in range(N_EX):
            xs = x_pool.tile([128, N_T, D], BF16, tag="x")
            for q in range(4):
                nc.sync.dma_start(xs[:, 8 * q:8 * (q + 1), :],
                                  x_d[e, :, 8 * q:8 * (q + 1), :])
            x_tiles.append(xs)

        st = [dict() for _ in range(N_EX)]

        # one shared PSUM bank subdivided for the small per-round tiles.
        # layout (f32 columns): yT @0..32, accT @32..64, rbc @64..96,
        # wsps @96..128, nrow @128..144, acc(j2 only) @224..480 (parts 0..15)
        def misc_views():
            m = ps_m.tile([128, 480], F32, tag="misc")
            yT_ps = m[:, 0:32].rearrange("p (c n) -> p c n", n=N)
            accT_ps = m[:, 32:64].rearrange("p (c n) -> p c n", n=N)
            rbc_ps = m[:, 64:96].rearrange("p (c n) -> p c n", n=N)
            wsps = m[:, 96:128].rearrange("p (c n) -> p c n", n=N)
            nrow_ps = m[0:1, 128:144]
            acc_ps = m[0:N, 224:480]
            return yT_ps, accT_ps, rbc_ps, wsps, nrow_ps, acc_ps

        # big-copy engine rotation: P7 V5 A4 per 16
        cp_engines = [nc.gpsimd, nc.vector, nc.scalar, nc.gpsimd,
                      nc.vector, nc.gpsimd, nc.scalar, nc.vector,
                      nc.gpsimd, nc.gpsimd, nc.vector, nc.scalar,
                      nc.gpsimd, nc.vector, nc.gpsimd, nc.scalar]

        def phase_T(e):
            """Build xT[d, i] via PE transposes; fold in colsum matmuls
            (round-0 shortcut: c uniform -> yT0 = colsum(x)/16)."""
            xs = x_tiles[e]
            xT = xT_pool.tile([128, 2, IN], BF16, tag="xT")
            cs = misc_views()
            cs_ps = cs[0]  # yT slot of this misc buf
            for tp in range(N_T // 2):
                psT = ps_t.tile([128, 2, 2, 128], BF16, tag="psT")
                for ti in range(2):
                    t = 2 * tp + ti
                    for dc in range(2):
                        nc.tensor.transpose(
                            psT[:, dc, ti, :],
                            xs[:, t, 128 * dc:128 * (dc + 1)], id128[:])
                        nc.tensor.matmul(
                            cs_ps[:, dc, 0:1],
                            xs[:, t, 128 * dc:128 * (dc + 1)], ones128[:],
                            start=(t == 0), stop=(t == N_T - 1),
                            skip_group_check=True)
                eng = cp_engines[tp]
                dst = xT[:, :, 256 * tp:256 * (tp + 1)]
                if eng is nc.vector:
                    eng.tensor_copy(dst, psT[:])
                elif eng is nc.scalar:
                    eng.copy(dst.bitcast(F32), psT[:].bitcast(F32))
                else:
                    eng.tensor_copy(dst.bitcast(F32), psT[:].bitcast(F32))
            st[e]["xT"] = xT
            st[e]["cs_ps"] = cs_ps

        def phase_J(e, j):
            """One routing round: yT -> accT -> squash -> WS -> b ->
            softmax (j<2) or final output (j=2)."""
            xT = st[e]["xT"]
            xs = x_tiles[e]
            c_all = st[e].get("c_all")
            yT_ps, accT_ps, rbc_ps, wsps, nrow_ps, acc_ps = misc_views()

            # --- yT[d, n] = sum_i x[i, d] c[i, n] ---
            if j == 0:
                yT_sb = small_pool.tile([128, 2, N], BF16, tag="yTsb")
                nc.vector.tensor_scalar_mul(
                    yT_sb[:],
                    st[e]["cs_ps"][:, :, 0:1].to_broadcast([128, 2, N]),
                    1.0 / N)
            else:
                for t in range(N_T):
                    for dc in range(2):
                        nc.tensor.matmul(
                            yT_ps[:, dc, :],
                            xs[:, t, 128 * dc:128 * (dc + 1)], c_all[:, t, :],
                            start=(t == 0), stop=(t == N_T - 1),
                            skip_group_check=True)
                yT_sb = small_pool.tile([128, 2, N], BF16, tag="yTsb")
                nc.gpsimd.tensor_copy(yT_sb[:], yT_ps[:])

            if j == 2:
                # final round: plain acc [16, 256], compact extraction
                for dc in range(2):
                    nc.tensor.matmul(acc_ps[:], yT_sb[:, dc, :], Wt[:, dc, :],
                                     start=(dc == 0), stop=(dc == 1),
                                     skip_group_check=True)
                om = small_pool.tile([N, M], BF16, tag="om")
                nc.gpsimd.tensor_mul(om[:], acc_ps[:], bmask[:])
                sq = small_pool.tile([N, M], BF16, tag="sq")
                nrm2 = small_pool.tile([N, 1], F32, tag="nrm2")
                nc.scalar.activation(sq[:], om[:], AF.Square,
                                     accum_out=nrm2[:])
                lnv = small_pool.tile([N, 1], F32, tag="lnv")
                nc.scalar.activation(lnv[:], nrm2[:], AF.Ln, bias=eps_t[:])
                rinv = small_pool.tile([N, 1], F32, tag="rinv")
                nc.scalar.activation(rinv[:], lnv[:], AF.Exp, scale=-0.5)
                o_c = small_pool.tile([N, K], F32, tag="o_c")
                nc.vector.tensor_reduce(
                    o_c[:], om[:].rearrange("p (g k) -> p k g", k=K),
                    axis=mybir.AxisListType.X, op=mybir.AluOpType.add)
                nc.vector.tensor_scalar_mul(out_stage[:, e, :], o_c[:],
                                            rinv[:])
                return

            # --- accT[m, n] = sum_d W[d, m] yT[d, n] (transposed form) ---
            for mc in range(2):
                for dc in range(2):
                    nc.tensor.matmul(
                        accT_ps[:, mc, :],
                        Wt[:, dc, 128 * mc:128 * (mc + 1)], yT_sb[:, dc, :],
                        start=(dc == 0), stop=(dc == 1),
                        skip_group_check=True)
            omT = small_pool.tile([128, 2, N], BF16, tag="omT")
            nc.gpsimd.tensor_mul(omT[:], accT_ps[:], bmaskT[:])

            # branch 1 (rinv): sqT -> nrow -> ln -> exp -> rbc
            sqT = small_pool.tile([128, 2, N], BF16, tag="sqT")
            nc.vector.tensor_mul(sqT[:], omT[:], omT[:])
            for mc in range(2):
                nc.tensor.matmul(nrow_ps[:], ones128[:], sqT[:, mc, :],
                                 start=(mc == 0), stop=(mc == 1),
                                 skip_group_check=True)
            lnr = small_pool.tile([1, N], F32, tag="lnr")
            nc.scalar.activation(lnr[:], nrow_ps[:], AF.Ln, bias=eps_r[:])
            rrow = small_pool.tile([1, N], BF16, tag="rrow")
            nc.scalar.activation(rrow[:], lnr[:], AF.Exp, scale=-0.5)
            for mc in range(2):
                nc.tensor.matmul(rbc_ps[:, mc, :], onesrow[:], rrow[:],
                                 start=True, stop=True,
                                 skip_group_check=True)

            # branch 2 (runs in parallel): WS_raw = W @ omT; rinv scales
            # out of the m-contraction, applied at the ws copy
            for dc in range(2):
                for mc in range(2):
                    nc.tensor.matmul(
                        wsps[:, dc, :],
                        WtT[:, mc, 128 * dc:128 * (dc + 1)], omT[:, mc, :],
                        start=(mc == 0), stop=(mc == 1),
                        skip_group_check=True)
            ws = small_pool.tile([128, 2, N], BF16, tag="ws")
            nc.vector.tensor_mul(ws[:], wsps[:], rbc_ps[:])

            # --- b = x @ WS ---
            b_ps = ps_b.tile([128, N_T, N], F32, tag="b")
            for t in range(N_T):
                for dc in range(2):
                    nc.tensor.matmul(
                        b_ps[:, t, :],
                        xT[:, dc, 128 * t:128 * (t + 1)], ws[:, dc, :],
                        start=(dc == 0), stop=(dc == 1),
                        skip_group_check=True)

            # --- softmax over n, pipelined in i-halves so the next round's
            # yT matmuls for tiles 0..15 start after only half the tail ---
            e_all = c_pool.tile([128, N_T, N], BF16, tag="e_all")
            s_sum = c_pool.tile([128, N_T], F32, tag="s_sum")
            s_r = c_pool.tile([128, N_T], F32, tag="s_r")
            c_new = c_pool.tile([128, N_T, N], BF16, tag="c_all")
            H = N_T // 2
            for h in range(2):
                sl = slice(H * h, H * (h + 1))
                nc.scalar.activation(e_all[:, sl, :], b_ps[:, sl, :], AF.Exp)
                nc.vector.tensor_reduce(s_sum[:, sl], e_all[:, sl, :],
                                        axis=mybir.AxisListType.X,
                                        op=mybir.AluOpType.add)
                nc.vector.reciprocal(s_r[:, sl], s_sum[:, sl])
                nc.vector.tensor_mul(
                    c_new[:, sl, :], e_all[:, sl, :],
                    s_r[:, sl].to_broadcast([128, H, N]))
            st[e]["c_all"] = c_new

        # wavefront over examples to keep PE dense across dependency stalls
        order = [(0, "T"), (0, 0), (1, "T"), (0, 1), (1, 0), (2, "T"),
                 (0, 2), (1, 1), (2, 0), (3, "T"), (1, 2), (2, 1), (3, 0),
                 (2, 2), (3, 1), (3, 2)]
        for e, ph in order:
            PHASE_MARKS.append(((e, ph), nc.next_id()))
            if ph == "T":
                phase_T(e)
            else:
                phase_J(e, ph)
        PHASE_MARKS.append((("end", 0), nc.next_id()))

        nc.sync.dma_start(out_d.ap().rearrange("e n k -> n e k"), out_stage[:])

    nc.compile()
    return nc


PHASE_MARKS = []

_NC_CACHE = {}


def _get_nc():
    if "nc" not in _NC_CACHE:
        _NC_CACHE["nc"] = build_kernel()
    return _NC_CACHE["nc"]


def make_const_inputs():
    bf = ml_dtypes.bfloat16
    id128 = np.eye(128, dtype=bf)
    ones128 = np.ones((128, 1), dtype=bf)
    bmask = np.zeros((N, M), dtype=np.float32)
    for n in range(N):
        bmask[n, n * K:(n + 1) * K] = 1.0
    # bmaskT[p, mc, n] = bmask[n, mc*128 + p]
    bmaskT = np.ascontiguousarray(
        bmask.T.reshape(2, 128, N).transpose(1, 0, 2))
    return id128, ones128, bmask.astype(bf), bmaskT.astype(bf)


def kernel(x, W, num_capsule=None, dim_capsule=None, routings=None, **_):
    bf = ml_dtypes.bfloat16
    x = np.asarray(x, dtype=np.float32)
    W = np.asarray(W, dtype=np.float32)
    assert x.shape == (B, IN, D), x.shape

    nc = _get_nc()
    id128, ones128, bmask, bmaskT = make_const_inputs()
    W0 = W[0]
    Wt = np.ascontiguousarray(
        W0.reshape(2, 128, M).transpose(1, 0, 2)).astype(bf)
    WtT = np.ascontiguousarray(
        W0.T.reshape(2, 128, D).transpose(1, 0, 2)).astype(bf)

    # x[b, i, d] -> [core, e, p, t, d] with i = t*128 + p
    xr = np.ascontiguousarray(
        x.reshape(N_CORES, N_EX, N_T, 128, D).transpose(0, 1, 3, 2, 4)
    ).astype(bf)

    in_maps = []
    for c in range(N_CORES):
        in_maps.append({"x": xr[c], "Wt": Wt, "WtT": WtT, "id128": id128,
                        "ones128": ones128, "bmask": bmask,
                        "bmaskT": bmaskT})

    res = run_bass_kernel_spmd(nc, in_maps, core_ids=list(range(N_CORES)))
    out = np.concatenate([r["out"] for r in res.results], axis=0)
    return out.astype(np.float32)


# revision 43
# speedup vs baseline: 1.1652x; 1.1021x over previous
"""Capsule-routing kernel for Trainium2, 8-core batch-parallel.

Reference computation (per example, In=4096, D=256, N=16, K=16, routings=3):
    u_hat = (x @ W).reshape(In, N, K)           # [In, 256] with m = n*16+k
    b = 0
    for j in range(3):
        c = softmax(b, axis=n)                   # [In, N]
        outputs = squash(sum_i c[i,n] u_hat[i,n,:])   # [N, K]
        if j < 2: b[i,n] = sum_k outputs[n,k] u_hat[i,n,k]

Key algebraic restructuring: u_hat is NEVER materialized.
  - outputs accumulation:  acc = (c^T x) W = yT^T @ W  with
    yT[d,n] = sum_i x[i,d] c[i,n]  (64 matmuls of 16-free per round)
  - b update:  b = (x W) S = x @ (W S)  with WS [256,16] built by 4 tiny
    matmuls from S = (masked outputs)^T scaled by rinv.
  - round 0 (c uniform 1/16): yT0 = colsum(x)/16 broadcast, where
    colsum accumulates via 1-col matmuls folded into the transpose phase.
  - squash: om = acc*mask is transposed on PE; Square runs on the
    128-partition form (32-free); nrm2 via PE matmul with ones;
    rinv = exp(-0.5*ln(nrm2+eps)) so ScalarE stays on ONE ACT table
    (natural_log_exp_and_others: Copy/Square/Exp/Ln).
x is shipped bf16 i-major; xT (lhsT for the b update) is built on
device with PE transposes. Everything runs in bf16 (tol 2e-2).
"""

import sys
from contextlib import ExitStack

sys.path.insert(0, "/opt/trn_rl_repo")

import numpy as np
import ml_dtypes

import concourse.bass as bass
import concourse.mybir as mybir
import concourse.tile as tile
from concourse import bacc
from concourse.bass_utils import run_bass_kernel_spmd

# All ScalarE funcs used here (Copy/Square/Exp/Ln) live together in the
# natural_log_exp_and_others ACT table. Put it first so the act-table
# insertion pass resolves every activation to that one table instead of
# thrashing between exp_and_others and natural_log (1283 ns per reload).
from concourse.hw_specs import get_activation_tables as _gat_orig


def _gat_pref(arch):
    t = _gat_orig(arch)
    pref = "natural_log_exp_and_others"
    if pref in t:
        return {pref: t[pref],
                **{k: v for k, v in t.items() if k != pref}}
    return t


bacc.get_activation_tables = _gat_pref

F32 = mybir.dt.float32
BF16 = mybir.dt.bfloat16
AF = mybir.ActivationFunctionType

N_CORES = 8
B = 32
IN = 4096
D = 256
N = 16
K = 16
M = N * K  # 256
EPS = 1e-7
N_EX = 4
N_T = 32  # i tiles of 128


def build_kernel():
    nc = bacc.Bacc("TRN2", target_bir_lowering=False, debug=False,
                   num_devices=N_CORES)

    x_d = nc.dram_tensor("x", [N_EX, 128, N_T, D], BF16, kind="ExternalInput")
    Wt_d = nc.dram_tensor("Wt", [128, 2, M], BF16, kind="ExternalInput")
    WtT_d = nc.dram_tensor("WtT", [128, 2, D], BF16, kind="ExternalInput")
    id128_d = nc.dram_tensor("id128", [128, 128], BF16, kind="ExternalInput")
    ones_d = nc.dram_tensor("ones128", [128, 1], BF16, kind="ExternalInput")
    bmask_d = nc.dram_tensor("bmask", [N, M], BF16, kind="ExternalInput")
    bmaskT_d = nc.dram_tensor("bmaskT", [128, 2, N], BF16,
                              kind="ExternalInput")
    out_d = nc.dram_tensor("out", [N_EX, N, K], F32, kind="ExternalOutput")

    with tile.TileContext(nc) as tc, ExitStack() as ctx:
        const_pool = ctx.enter_context(tc.tile_pool(name="consts", bufs=1))
        x_pool = ctx.enter_context(tc.tile_pool(name="x", bufs=4))
        xT_pool = ctx.enter_context(tc.tile_pool(name="xT", bufs=4))
        c_pool = ctx.enter_context(tc.tile_pool(name="c", bufs=4))
        small_pool = ctx.enter_context(tc.tile_pool(name="small", bufs=4))

        ps_t = ctx.enter_context(tc.tile_pool(name="ps_t", bufs=2, space="PSUM"))
        ps_b = ctx.enter_context(tc.tile_pool(name="ps_b", bufs=2, space="PSUM"))
        ps_m = ctx.enter_context(tc.tile_pool(name="ps_m", bufs=4, space="PSUM"))

        # ---- constants ----
        Wt = const_pool.tile([128, 2, M], BF16, tag="Wt")
        nc.sync.dma_start(Wt[:], Wt_d[:])
        WtT = const_pool.tile([128, 2, D], BF16, tag="WtT")
        nc.sync.dma_start(WtT[:], WtT_d[:])
        id128 = const_pool.tile([128, 128], BF16, tag="id128")
        nc.sync.dma_start(id128[:], id128_d[:])
        ones128 = const_pool.tile([128, 1], BF16, tag="ones128")
        nc.sync.dma_start(ones128[:], ones_d[:])
        onesrow = const_pool.tile([1, 128], BF16, tag="onesrow")
        nc.sync.dma_start(onesrow[:], ones_d.ap().rearrange("p o -> o p"))
        bmask = const_pool.tile([N, M], BF16, tag="bmask")
        nc.sync.dma_start(bmask[:], bmask_d[:])
        bmaskT = const_pool.tile([128, 2, N], BF16, tag="bmaskT")
        nc.sync.dma_start(bmaskT[:], bmaskT_d[:])
        out_stage = const_pool.tile([N, N_EX, K], F32, tag="out_stage")
        eps_t = const_pool.tile([N, 1], F32, tag="eps")
        nc.vector.memset(eps_t[:], EPS)
        eps_r = const_pool.tile([1, 1], F32, tag="eps_r")
        nc.vector.memset(eps_r[:], EPS)

        # ---- x loads (prefetch, chunked so compute can start early) ----
        x_tiles = []
        for e in range(N_EX):
            xs = x_pool.tile([128, N_T, D], BF16, tag="x")
            for q in range(4):
                nc.sync.dma_start(xs[:, 8 * q:8 * (q + 1), :],
                                  x_d[e, :, 8 * q:8 * (q + 1), :])
            x_tiles.append(xs)

        st = [dict() for _ in range(N_EX)]

        # one shared PSUM bank subdivided for the small per-round tiles.
        # layout (f32 columns): yT @0..32, accT @32..64, rbc @64..96,
        # wsps @96..128, nrow @128..144, acc(j2 only) @224..480 (parts 0..15)
        def misc_views():
            m = ps_m.tile([128, 480], F32, tag="misc")
            yT_ps = m[:, 0:32].rearrange("p (c n) -> p c n", n=N)
            accT_ps = m[:, 32:64].rearrange("p (c n) -> p c n", n=N)
            rbc_ps = m[:, 64:96].rearrange("p (c n) -> p c n", n=N)
            wsps = m[:, 96:128].rearrange("p (c n) -> p c n", n=N)
            nrow_ps = m[0:1, 128:144]
            acc_ps = m[0:N, 224:480]
            return yT_ps, accT_ps, rbc_ps, wsps, nrow_ps, acc_ps

        # big-copy engine rotation (8 groups per example)
        cp_engines = [nc.vector, nc.gpsimd, nc.scalar, nc.vector,
                      nc.gpsimd, nc.scalar, nc.vector, nc.gpsimd]

        def phase_T(e):
            """Build xT[d, i] via PE transposes; fold in colsum matmuls
            (round-0 shortcut: c uniform -> yT0 = colsum(x)/16).
            Groups of 4 i-tiles per PSUM bank halve the copy round-trips."""
            xs = x_tiles[e]
            xT = xT_pool.tile([128, 2, IN], BF16, tag="xT")
            cs = misc_views()
            cs_ps = cs[0]  # yT slot of this misc buf
            for g in range(N_T // 4):
                psT = ps_t.tile([128, 2, 4, 128], BF16, tag="psT")
                for ti in range(4):
                    t = 4 * g + ti
                    for dc in range(2):
                        nc.tensor.transpose(
                            psT[:, dc, ti, :],
                            xs[:, t, 128 * dc:128 * (dc + 1)], id128[:])
                        nc.tensor.matmul(
                            cs_ps[:, dc, 0:1],
                            xs[:, t, 128 * dc:128 * (dc + 1)], ones128[:],
                            start=(t == 0), stop=(t == N_T - 1),
                            skip_group_check=True)
                eng = cp_engines[g]
                dst = xT[:, :, 512 * g:512 * (g + 1)]
                if eng is nc.vector:
                    eng.tensor_copy(dst, psT[:])
                elif eng is nc.scalar:
                    eng.copy(dst.bitcast(F32), psT[:].bitcast(F32))
                else:
                    eng.tensor_copy(dst.bitcast(F32), psT[:].bitcast(F32))
            st[e]["xT"] = xT
            st[e]["cs_ps"] = cs_ps

        def round_stages(e, j):
            """Stage-closures for routing round (e, j), for lockstep
            emission across examples (avoids head-of-line blocking in the
            in-order engine streams)."""
            xT = st[e]["xT"]
            xs = x_tiles[e]
            cx = {}

            def s_yT():
                cx["v"] = misc_views()
                if j == 0:
                    yT_sb = small_pool.tile([128, 2, N], BF16, tag="yTsb",
                                            name="yTsb")
                    nc.vector.tensor_scalar_mul(
                        yT_sb[:],
                        st[e]["cs_ps"][:, :, 0:1].to_broadcast([128, 2, N]),
                        1.0 / N)
                    cx["yT_sb"] = yT_sb
                else:
                    c_all = st[e]["c_all"]
                    yT_ps = cx["v"][0]
                    for t in range(N_T):
                        for dc in range(2):
                            nc.tensor.matmul(
                                yT_ps[:, dc, :],
                                xs[:, t, 128 * dc:128 * (dc + 1)],
                                c_all[:, t, :],
                                start=(t == 0), stop=(t == N_T - 1),
                                skip_group_check=True)

            def s_yTsb():
                if j != 0:
                    yT_sb = small_pool.tile([128, 2, N], BF16, tag="yTsb",
                                            name="yTsb")
                    nc.gpsimd.tensor_copy(yT_sb[:], cx["v"][0])
                    cx["yT_sb"] = yT_sb

            if j == 2:
                def s_acc():
                    acc_ps = cx["v"][5]
                    yT_sb = cx["yT_sb"]
                    for dc in range(2):
                        nc.tensor.matmul(acc_ps[:], yT_sb[:, dc, :],
                                         Wt[:, dc, :],
                                         start=(dc == 0), stop=(dc == 1),
                                         skip_group_check=True)

                def s_om():
                    om = small_pool.tile([N, M], BF16, tag="om", name="om")
                    nc.gpsimd.tensor_mul(om[:], cx["v"][5], bmask[:])
                    cx["om"] = om

                def s_sq():
                    sq = small_pool.tile([N, M], BF16, tag="sq", name="sq")
                    nrm2 = small_pool.tile([N, 1], F32, tag="nrm2",
                                           name="nrm2")
                    nc.scalar.activation(sq[:], cx["om"][:], AF.Square,
                                         accum_out=nrm2[:])
                    lnv = small_pool.tile([N, 1], F32, tag="lnv", name="lnv")
                    nc.scalar.activation(lnv[:], nrm2[:], AF.Ln,
                                         bias=eps_t[:])
                    rinv = small_pool.tile([N, 1], F32, tag="rinv",
                                           name="rinv")
                    nc.scalar.activation(rinv[:], lnv[:], AF.Exp, scale=-0.5)
                    cx["rinv"] = rinv

                def s_out():
                    o_c = small_pool.tile([N, K], F32, tag="o_c", name="o_c")
                    nc.vector.tensor_reduce(
                        o_c[:],
                        cx["om"][:].rearrange("p (g k) -> p k g", k=K),
                        axis=mybir.AxisListType.X, op=mybir.AluOpType.add)
                    nc.vector.tensor_scalar_mul(out_stage[:, e, :], o_c[:],
                                                cx["rinv"][:])

                return [s_yT, s_yTsb, s_acc, s_om, s_sq, s_out]

            def s_accT():
                accT_ps = cx["v"][1]
                yT_sb = cx["yT_sb"]
                for mc in range(2):
                    for dc in range(2):
                        nc.tensor.matmul(
                            accT_ps[:, mc, :],
                            Wt[:, dc, 128 * mc:128 * (mc + 1)],
                            yT_sb[:, dc, :],
                            start=(dc == 0), stop=(dc == 1),
                            skip_group_check=True)
                omT = small_pool.tile([128, 2, N], BF16, tag="omT",
                                      name="omT")
                nc.gpsimd.tensor_mul(omT[:], accT_ps[:], bmaskT[:])
                cx["omT"] = omT

            def s_sqT():
                omT = cx["omT"]
                sqT = small_pool.tile([128, 2, N], BF16, tag="sqT",
                                      name="sqT")
                nc.vector.tensor_mul(sqT[:], omT[:], omT[:])
                nrow_ps = cx["v"][4]
                for mc in range(2):
                    nc.tensor.matmul(nrow_ps[:], ones128[:], sqT[:, mc, :],
                                     start=(mc == 0), stop=(mc == 1),
                                     skip_group_check=True)
                # WS_raw in parallel with the rinv chain
                wsps = cx["v"][3]
                for dc in range(2):
                    for mc in range(2):
                        nc.tensor.matmul(
                            wsps[:, dc, :],
                            WtT[:, mc, 128 * dc:128 * (dc + 1)],
                            omT[:, mc, :],
                            start=(mc == 0), stop=(mc == 1),
                            skip_group_check=True)

            def s_lnexp():
                lnr = small_pool.tile([1, N], F32, tag="lnr", name="lnr")
                nc.scalar.activation(lnr[:], cx["v"][4], AF.Ln,
                                     bias=eps_r[:])
                rrow = small_pool.tile([1, N], BF16, tag="rrow", name="rrow")
                nc.scalar.activation(rrow[:], lnr[:], AF.Exp, scale=-0.5)
                rbc_ps = cx["v"][2]
                for mc in range(2):
                    nc.tensor.matmul(rbc_ps[:, mc, :], onesrow[:], rrow[:],
                                     start=True, stop=True,
                                     skip_group_check=True)

            def s_ws():
                ws = small_pool.tile([128, 2, N], BF16, tag="ws", name="ws")
                nc.vector.tensor_mul(ws[:], cx["v"][3], cx["v"][2])
                cx["ws"] = ws

            def s_b():
                b_ps = ps_b.tile([128, N_T, N], F32, tag="b", name="b_ps")
                ws = cx["ws"]
                for t in range(N_T):
                    for dc in range(2):
                        nc.tensor.matmul(
                            b_ps[:, t, :],
                            xT[:, dc, 128 * t:128 * (t + 1)], ws[:, dc, :],
                            start=(dc == 0), stop=(dc == 1),
                            skip_group_check=True)
                cx["b_ps"] = b_ps
                cx["e_all"] = c_pool.tile([128, N_T, N], BF16, tag="e_all",
                                          name="e_all")
                cx["s_sum"] = c_pool.tile([128, N_T], F32, tag="s_sum",
                                          name="s_sum")
                cx["s_r"] = c_pool.tile([128, N_T], F32, tag="s_r",
                                        name="s_r")
                cx["c_new"] = c_pool.tile([128, N_T, N], BF16, tag="c_all",
                                          name="c_new")
                st[e]["c_all"] = cx["c_new"]

            def softmax_half(h):
                H = N_T // 2
                sl = slice(H * h, H * (h + 1))
                nc.scalar.activation(cx["e_all"][:, sl, :],
                                     cx["b_ps"][:, sl, :], AF.Exp)
                nc.vector.tensor_reduce(cx["s_sum"][:, sl],
                                        cx["e_all"][:, sl, :],
                                        axis=mybir.AxisListType.X,
                                        op=mybir.AluOpType.add)
                nc.vector.reciprocal(cx["s_r"][:, sl], cx["s_sum"][:, sl])
                nc.vector.tensor_mul(
                    cx["c_new"][:, sl, :], cx["e_all"][:, sl, :],
                    cx["s_r"][:, sl].to_broadcast([128, H, N]))

            return [s_yT, s_yTsb, s_accT, s_sqT, s_lnexp, s_ws, s_b,
                    lambda: softmax_half(0), lambda: softmax_half(1)]

        def emit_round(members):
            """Emit rounds for `members` [(e, j), ...] stage-locked."""
            stage_lists = [round_stages(e, j) for e, j in members]
            nst = max(len(s) for s in stage_lists)
            for si in range(nst):
                for sl in stage_lists:
                    if si < len(sl):
                        sl[si]()

        # Load window: T(e) then its round-0 immediately (chain overlaps
        # the next example's DMA + transposes). J1/J2 after all loads,
        # stage-locked across all four examples.
        for e in range(N_EX):
            PHASE_MARKS.append(((e, "T"), nc.next_id()))
            phase_T(e)
            PHASE_MARKS.append(((e, 0), nc.next_id()))
            emit_round([(e, 0)])
        PHASE_MARKS.append((("J1", 0), nc.next_id()))
        emit_round([(e, 1) for e in range(N_EX)])
        PHASE_MARKS.append((("J2", 0), nc.next_id()))
        emit_round([(e, 2) for e in range(N_EX)])
        PHASE_MARKS.append((("end", 0), nc.next_id()))

        nc.sync.dma_start(out_d.ap().rearrange("e n k -> n e k"), out_stage[:])

    nc.compile()
    return nc


PHASE_MARKS = []

_NC_CACHE = {}


def _get_nc():
    if "nc" not in _NC_CACHE:
        _NC_CACHE["nc"] = build_kernel()
    return _NC_CACHE["nc"]


def make_const_inputs():
    bf = ml_dtypes.bfloat16
    id128 = np.eye(128, dtype=bf)
    ones128 = np.ones((128, 1), dtype=bf)
    bmask = np.zeros((N, M), dtype=np.float32)
    for n in range(N):
        bmask[n, n * K:(n + 1) * K] = 1.0
    # bmaskT[p, mc, n] = bmask[n, mc*128 + p]
    bmaskT = np.ascontiguousarray(
        bmask.T.reshape(2, 128, N).transpose(1, 0, 2))
    return id128, ones128, bmask.astype(bf), bmaskT.astype(bf)


def kernel(x, W, num_capsule=None, dim_capsule=None, routings=None, **_):
    bf = ml_dtypes.bfloat16
    x = np.asarray(x, dtype=np.float32)
    W = np.asarray(W, dtype=np.float32)
    assert x.shape == (B, IN, D), x.shape

    nc = _get_nc()
    id128, ones128, bmask, bmaskT = make_const_inputs()
    W0 = W[0]
    Wt = np.ascontiguousarray(
        W0.reshape(2, 128, M).transpose(1, 0, 2)).astype(bf)
    WtT = np.ascontiguousarray(
        W0.T.reshape(2, 128, D).transpose(1, 0, 2)).astype(bf)

    # x[b, i, d] -> [core, e, p, t, d] with i = t*128 + p
    xr = np.ascontiguousarray(
        x.reshape(N_CORES, N_EX, N_T, 128, D).transpose(0, 1, 3, 2, 4)
    ).astype(bf)

    in_maps = []
    for c in range(N_CORES):
        in_maps.append({"x": xr[c], "Wt": Wt, "WtT": WtT, "id128": id128,
                        "ones128": ones128, "bmask": bmask,
                        "bmaskT": bmaskT})

    res = run_bass_kernel_spmd(nc, in_maps, core_ids=list(range(N_CORES)))
    out = np.concatenate([r["out"] for r in res.results], axis=0)
    return out.astype(np.float32)
